# revision 45
# baseline (speedup 1.0000x reference)
"""Trainium2 Bass kernel for EnhancedCapsuleLayer.

Math (per batch b):
  u_hat[n,c,h] = x[n,:] @ W[c,:,h]          (b_caps == 0 for this problem)
  routing(3 iters): c_i = softmax(blog, axis=c); s = sum_n c_i*u_hat;
                    v = squash(s); blog += u_hat . v
  then MHA self-attention over routed [C,H], residual, layernorm, squash*gamma.

Key factorization (u_hat is never materialized):
  s[c,h]   = sum_d xc[c,d] W[c,d,h],   xc = c_i^T x   (contract n)
  agr[n,c] = sum_d x[n,d] Wv[c,d],     Wv[c,d] = sum_h W[c,d,h] v[c,h]

Perf structure (vs the 80.6us baseline):
  - input DMA split across sync+scalar queues in consumption order
    (xbar, ws, wvw, xt, xn) so compute starts ~10us earlier.
  - blog lives in PSUM: routing-iteration-1 agreement matmuls accumulate
    onto iteration-0's (start=False), eliminating the blog tile and all
    DVE adds/copies for it.
  - squash never transposes: s stays in the pair-packed [i*64+h, 4q+2b+i]
    PSUM layout; sum-of-squares via a PE column-sum against ones, the
    scale chain is 10 fused DVE ops, and the scale is broadcast back with
    a K=1 ones-outer-product matmul.  The *unscaled* Wv matmuls run in
    parallel with the scale chain; the scale is applied in the single
    PSUM->SBUF copy of Wv^T.
  - Wv via 32 N=4 matmuls against a [i*64+h, d] weight layout (wvw)
    producing Wv^T directly in the layout agr consumes (no transposes).
  - MHA is transpose-free: q/k/v projections consume the pair-packed v
    directly (weights replicated over the i-halves), scores are computed
    transposed so softmax normalizes over the partition axis via PE
    column-sum + K=1 broadcast, context and output projection follow in
    the transposed layout.  LN uses bn_stats/bn_aggr; both rsqrts are
    fused Quake chains.

Sharding: data-parallel over batch (2 batches per core, 8 cores).
"""

import numpy as np
import ml_dtypes

B, N, D, C, H = 16, 2048, 256, 32, 64
HEADS, KD = 4, 64
NCORES = 8
BL = B // NCORES          # 2 batches per core
NT = N // 128             # 16 n-tiles
DC = D // 128             # 2 d-chunks
NPAIR = C // 2            # 16 capsule pairs
EPS_SQ = 1e-7
LN_EPS = 1e-3

_CACHE = {}


class _StageCut(Exception):
    pass


def _build(gamma_val):
    import os as _os
    STAGE = int(_os.environ.get("KBISECT_STAGE", "99"))
    KDUMP = _os.environ.get("KDUMP", "")
    import concourse.bass as bass
    import concourse.bacc as bacc
    import concourse.mybir as mybir
    import concourse.tile as tile

    f32 = mybir.dt.float32
    bf16 = mybir.dt.bfloat16
    i32 = mybir.dt.int32
    AX = mybir.AxisListType
    OP = mybir.AluOpType
    AF = mybir.ActivationFunctionType
    PSUM = bass.MemorySpace.PSUM

    nc = bacc.Bacc("TRN2", target_bir_lowering=False, debug=False)

    xn_d = nc.dram_tensor("xn", [128, BL, NT, 256], bf16, kind="ExternalInput")
    xt_d = nc.dram_tensor("xt", [128, BL, DC, N], bf16, kind="ExternalInput")
    xbar_d = nc.dram_tensor("xbar", [128, BL * DC], f32, kind="ExternalInput")
    ws_d = nc.dram_tensor("ws", [128, NPAIR * DC * 128], bf16, kind="ExternalInput")
    wvw_d = nc.dram_tensor("wvw", [128, NPAIR * DC * 128], bf16, kind="ExternalInput")
    wqk2_d = nc.dram_tensor("wqk2", [128, 512], bf16, kind="ExternalInput")
    wvv2_d = nc.dram_tensor("wvv2", [128, 256], bf16, kind="ExternalInput")
    wob2_d = nc.dram_tensor("wob2", [64, 256], bf16, kind="ExternalInput")
    i128b_d = nc.dram_tensor("i128b", [128, 128], bf16, kind="ExternalInput")
    out_d = nc.dram_tensor("out", [BL, C, H], f32, kind="ExternalOutput")
    if KDUMP:
        dbg32_d = nc.dram_tensor("dbg32", [128, 1024], f32, kind="ExternalOutput")
        dbg16_d = nc.dram_tensor("dbg16", [128, 1024], bf16, kind="ExternalOutput")

    MAGIC = 0x5F3759DF

    with tile.TileContext(nc) as tc:
      try:
        with (
            tc.tile_pool(name="const", bufs=1) as kc,
            tc.tile_pool(name="state", bufs=1) as ks,
            tc.tile_pool(name="work", bufs=2) as kw,
            tc.tile_pool(name="ps", bufs=1, space=PSUM) as pp,
            tc.tile_pool(name="pt", bufs=2, space=PSUM) as pt,
        ):
            # ---------------- DMA loads, in consumption order ----------------
            # sync and scalar queues each carry half of every big tensor so
            # both HBM queues stream at full rate; order within each queue is
            # first-use order: xbar -> ws -> wvw -> xt -> xn.
            # One queue, full-width tensors (128 partitions, >=8KB rows),
            # strictly in consumption order — a single HWDGE queue fans out
            # to all 16 DMA engines, and head-of-line priority is exactly
            # what the critical path wants.
            ws_sb = kc.tile([128, NPAIR * DC * 128], bf16, tag="ws_sb")
            ws_v = ws_sb[:].rearrange("p (q c m) -> p q c m", q=NPAIR, c=DC)
            nc.sync.dma_start(ws_sb[:], ws_d.ap())

            xbar_sb = kc.tile([128, BL * DC], f32, tag="xbar_sb")
            nc.sync.dma_start(xbar_sb[:], xbar_d.ap())

            wvw_sb = kc.tile([128, NPAIR * DC * 128], bf16, tag="wvw_sb")
            wvw_v = wvw_sb[:].rearrange("p (q c m) -> p q c m", q=NPAIR, c=DC)
            nc.sync.dma_start(wvw_sb[:], wvw_d.ap())

            xT_sb = kc.tile([128, BL * DC * N], bf16, tag="xT_sb")
            xT_v = xT_sb[:].rearrange("p (b c n) -> p b c n", b=BL, c=DC)
            xt_src = xt_d.ap()
            for b in range(BL):
                nc.sync.dma_start(xT_v[:, b], xt_src[:, b])

            x_sb = kc.tile([128, BL * NT * 256], bf16, tag="x_sb")
            x_v = x_sb[:].rearrange("p (b t d) -> p b t d", b=BL, t=NT)
            xn_src = xn_d.ap()
            for b in range(BL):
                nc.sync.dma_start(x_v[:, b], xn_src[:, b])

            # memsets first: they are pure engine ops and several gate the
            # first squash (ones_col) — don't queue them behind DMA enqueues.
            ones_col = kc.tile([128, 1], f32, tag="ones_col")
            nc.gpsimd.memset(ones_col[:], 1.0)
            ones_row = kc.tile([1, 128], f32, tag="ones_row")
            nc.gpsimd.memset(ones_row[:], 1.0)
            ones_b16 = kc.tile([128, 1], bf16, tag="ones_b16")
            nc.gpsimd.memset(ones_b16[:], 1.0)
            ones_rowb = kc.tile([1, 128], bf16, tag="ones_rowb")
            nc.gpsimd.memset(ones_rowb[:], 1.0)

            # MHA constants are first needed ~45us in: queue them BEHIND the
            # x loads on the same sync queue so they don't steal front BW.
            i128b = kc.tile([128, 128], bf16, tag="i128b")
            nc.sync.dma_start(i128b[:], i128b_d.ap())
            wqk2_sb = kc.tile([128, 512], bf16, tag="wqk2_sb")
            nc.sync.dma_start(wqk2_sb[:], wqk2_d.ap())
            wvv2_sb = kc.tile([128, 256], bf16, tag="wvv2_sb")
            nc.sync.dma_start(wvv2_sb[:], wvv2_d.ap())
            wob2_sb = kc.tile([64, 256], bf16, tag="wob2_sb")
            nc.sync.dma_start(wob2_sb[:], wob2_d.ap())

            # ---------------- state tiles ----------------
            xcT = ks.tile([128, DC * BL * C], bf16, tag="xcT")
            xcT_v = xcT[:].rearrange("p (c b q) -> p c b q", c=DC, b=BL)
            e_sb = ks.tile([128, BL * NT * C], bf16, tag="e_sb")
            e_v = e_sb[:].rearrange("p (b t c) -> p b t c", b=BL, t=NT)
            c_all = ks.tile([128, BL * NT * C], bf16, tag="c_all")
            c_v = c_all[:].rearrange("p (b t c) -> p b t c", b=BL, t=NT)
            ssum = ks.tile([128, BL * NT], f32, tag="ssum")
            ssum_v = ssum[:].rearrange("p (b t) -> p b t", b=BL)
            rs = ks.tile([128, BL * NT], f32, tag="rs")
            rs_v = rs[:].rearrange("p (b t) -> p b t", b=BL)
            sq = ks.tile([128, 64], f32, tag="sq")
            nc.gpsimd.memset(sq[:], 0.0)
            vblk4 = ks.tile([128, NPAIR * 4], bf16, tag="vblk4")
            vblk4_v = vblk4[:].rearrange("p (q m) -> p q m", q=NPAIR)
            nc.gpsimd.memset(vblk4[:], 0.0)
            v128 = ks.tile([128, 64], bf16, tag="v128")
            nc.gpsimd.memset(v128[:], 0.0)
            wvT = ks.tile([128, DC * 64], bf16, tag="wvT")
            sclbc_sb = ks.tile([128, 64], f32, tag="sclbc_sb")
            vbc = ks.tile([64, 64], f32, tag="vbc")

            # psum tiles (8 banks: 6 tagged + 2 in the pt pool)
            su3 = pp.tile([128, 64], f32, tag="su3")
            xcpd = [
                pp.tile([128, DC * C], f32, tag=f"xcpd{b}", name=f"xcpd{b}")
                for b in range(BL)
            ]
            agrp = [
                pp.tile([128, NT * C], f32, tag=f"agrp{b}", name=f"agrp{b}")
                for b in range(BL)
            ]
            wvp2 = pp.tile([128, DC * 64], f32, tag="wvp2")

            def dump32(name, ap, rows, cols):
                if KDUMP == name:
                    stg = kw.tile([rows, cols], f32, tag="dbg_stg")
                    nc.vector.tensor_copy(stg[:], ap)
                    nc.sync.dma_start(dbg32_d.ap()[0:rows, 0:cols], stg[:])
                    raise _StageCut()

            def dump16(name, ap, rows, cols):
                if KDUMP == name:
                    nc.sync.dma_start(dbg16_d.ap()[0:rows, 0:cols], ap)
                    raise _StageCut()

            def diag_ap(t, base_part, col_off, pitch, dims):
                a = t[:]
                return bass.AP(a.tensor, a.offset + base_part * pitch + col_off, dims)

            # su3 diagonal: partition 64i+h, col 4q+2b+i  (i = capsule parity)
            def su3_diag(i):
                return diag_ap(su3, 64 * i, i, 64, [[64, 64], [4, NPAIR], [2, 2]])

            def sq_diag(i):
                return diag_ap(sq, 64 * i, i, 64, [[64, 64], [4, NPAIR], [2, 2]])

            def vblk4_diag(i):
                return diag_ap(
                    vblk4, 64 * i, i, NPAIR * 4, [[NPAIR * 4, 64], [4, NPAIR], [2, 2]]
                )

            def quake_rsqrt(z, nm, p):
                # y ~= 1/sqrt(z), z an SBUF f32 AP shaped [p, w]; 5 DVE ops.
                w = z.shape[-1]
                t = kw.tile([p, w], i32, tag=f"qk_t{nm}", name=f"qk_t{nm}")
                nc.vector.tensor_scalar(
                    out=t[:], in0=z.bitcast(i32), scalar1=1, scalar2=-1,
                    op0=OP.arith_shift_right, op1=OP.bitwise_xor,
                )
                y = kw.tile([p, w], f32, tag=f"qk_y{nm}", name=f"qk_y{nm}")
                nc.vector.tensor_scalar(
                    out=y[:].bitcast(i32), in0=t[:], scalar1=MAGIC + 1,
                    scalar2=None, op0=OP.add,
                )
                a = kw.tile([p, w], f32, tag=f"qk_a{nm}", name=f"qk_a{nm}")
                nc.vector.tensor_mul(a[:], y[:], y[:])
                nc.vector.scalar_tensor_tensor(
                    out=a[:], in0=a[:], scalar=-0.5, in1=z,
                    op0=OP.mult, op1=OP.mult,
                )
                nc.vector.scalar_tensor_tensor(
                    out=y[:], in0=a[:], scalar=1.5, in1=y[:],
                    op0=OP.add, op1=OP.mult,
                )
                return y

            def squash_scale_row(s2p_ap, nm):
                # s2p_ap: PSUM [1, 64] sums of squares; returns SBUF [1,64]
                # scl = s2/(1+s2)/sqrt(s2+eps) = s2 * rsqrt((1+s2)^2 (s2+eps));
                # fused DVE ops only (the ACT Ln table lives in a different
                # act-func set than Exp and would thrash 1.5us table loads).
                a1 = kw.tile([1, 64], f32, tag=f"sc_a1{nm}", name=f"sc_a1{nm}")
                nc.vector.tensor_scalar_add(a1[:], s2p_ap, 1.0)
                t2 = kw.tile([1, 64], f32, tag=f"sc_t2{nm}", name=f"sc_t2{nm}")
                nc.vector.tensor_mul(t2[:], a1[:], a1[:])
                z = kw.tile([1, 64], f32, tag=f"sc_z{nm}", name=f"sc_z{nm}")
                nc.vector.scalar_tensor_tensor(
                    out=z[:], in0=s2p_ap, scalar=EPS_SQ, in1=t2[:],
                    op0=OP.add, op1=OP.mult,
                )
                y = quake_rsqrt(z[:], nm, 1)
                scl = kw.tile([1, 64], f32, tag=f"sc_s{nm}", name=f"sc_s{nm}")
                nc.vector.tensor_tensor(out=scl[:], in0=s2p_ap, in1=y[:], op=OP.mult)
                return scl

            TG = 4
            for it in range(3):
                if STAGE < 10 * it + 2:
                    break
                # ---------- routing coefficients + xc^T ----------
                if it == 0:
                    # uniform c = 1/C: xc[c, :] = xbar/C for every c
                    src = (
                        xbar_sb[:]
                        .rearrange("p (b c) -> p c b", b=BL)
                        .unsqueeze(-1)
                        .broadcast_to([128, DC, BL, C])
                    )
                    nc.vector.tensor_scalar(
                        out=xcT_v, in0=src, scalar1=1.0 / C, scalar2=None,
                        op0=OP.mult,
                    )
                    dump16("xcT_0", xcT[:], 128, DC * BL * C)
                else:
                    # chunked softmax straight out of the PSUM blog (= the
                    # accumulated agreements), pipelined against the PE.
                    for b in range(BL):
                        for tg in range(NT // TG):
                            t0 = tg * TG
                            sl = slice(t0 * C, (t0 + TG) * C)
                            nc.scalar.activation(
                                e_v[:, b, t0 : t0 + TG],
                                agrp[b][:, sl],
                                AF.Exp,
                            )
                            nc.vector.tensor_reduce(
                                out=ssum_v[:, b, t0 : t0 + TG],
                                in_=e_v[:, b, t0 : t0 + TG],
                                axis=AX.X, op=OP.add,
                            )
                            nc.vector.reciprocal(
                                rs_v[:, b, t0 : t0 + TG],
                                ssum_v[:, b, t0 : t0 + TG],
                            )
                            nc.vector.tensor_tensor(
                                out=c_v[:, b, t0 : t0 + TG],
                                in0=e_v[:, b, t0 : t0 + TG],
                                in1=rs_v[:, b, t0 : t0 + TG]
                                .unsqueeze(-1)
                                .broadcast_to([128, TG, C]),
                                op=OP.mult,
                            )
                    # xcT[d, c] directly: lhsT = x chunk (n-major), rhs = c.
                    for b in range(BL):
                        for dc in range(DC):
                            for t in range(NT):
                                nc.tensor.matmul(
                                    xcpd[b][:, dc * C : (dc + 1) * C],
                                    x_v[:, b, t, dc * 128 : (dc + 1) * 128],
                                    c_v[:, b, t, :],
                                    start=(t == 0),
                                    stop=(t == NT - 1),
                                )
                            nc.vector.tensor_copy(
                                xcT_v[:, dc, b, :],
                                xcpd[b][:, dc * C : (dc + 1) * C],
                            )
                    dump16(f"c_{it}", c_all[:], 128, BL * NT * C)
                    dump16(f"xcT_{it}", xcT[:], 128, DC * BL * C)

                # ---------- s = xc (*) W  (pair-packed) ----------
                # dc-outer with a single start: the dc0 half runs as soon as
                # the dc0 xcT copies land instead of stalling on dc1.
                if STAGE < 10 * it + 3:
                    break
                for dc in range(DC):
                    for q in range(NPAIR):
                        nc.tensor.matmul(
                            su3[:, 4 * q : 4 * q + 4],
                            ws_v[:, q, dc, :],
                            xcT_v[:, dc, :, 2 * q : 2 * q + 2],
                            start=(dc == 0 and q == 0),
                            stop=(dc == DC - 1),
                            skip_group_check=True,
                        )
                dump32(f"su3_{it}", su3[:], 128, 64)

                # ---------- squash scale chain (forked vs raw-Wv) ----------
                if STAGE < 10 * it + 4:
                    break
                # fork B first in PE issue order: the column-sum must not
                # queue behind the (possibly DMA-gated) Wv matmuls.
                for i in range(2):
                    nc.scalar.activation(sq_diag(i), su3_diag(i), AF.Square)
                s2p = pt.tile([1, 64], f32, tag="pt", name=f"s2p{it}")
                nc.tensor.matmul(s2p[:], ones_col[:], sq[:])

                def emit_wv_raw():
                    # raw (unscaled) Wv matmuls, overlapped with the scale chain
                    for i in range(2):
                        nc.vector.tensor_copy(vblk4_diag(i), su3_diag(i))
                    for q in range(NPAIR):
                        for dc in range(DC):
                            nc.tensor.matmul(
                                wvp2[:, dc * 64 + 4 * q : dc * 64 + 4 * q + 4],
                                wvw_v[:, q, dc, :],
                                vblk4_v[:, q, :],
                            )

                # At it0 the Wv matmuls are gated on the wvw DMA; emit the
                # broadcast first so the in-order PE doesn't queue it behind
                # them.  Later iterations flip the order so Wv runs during
                # the DVE scale chain.
                if it == 0:
                    scl = squash_scale_row(s2p[:], f"i{it}")
                    sclbc_p = pt.tile([128, 64], f32, tag="pt", name=f"sclbc{it}")
                    nc.tensor.matmul(sclbc_p[:], ones_row[:], scl[:])
                    emit_wv_raw()
                else:
                    if it < 2:
                        emit_wv_raw()
                    scl = squash_scale_row(s2p[:], f"i{it}")
                    sclbc_p = pt.tile([128, 64], f32, tag="pt", name=f"sclbc{it}")
                    nc.tensor.matmul(sclbc_p[:], ones_row[:], scl[:])
                nc.vector.tensor_copy(sclbc_sb[:], sclbc_p[:])
                dump32(f"scl_{it}", sclbc_sb[:], 128, 64)

                if it == 2:
                    break
                # join: scale the raw Wv^T during its PSUM->SBUF copy
                if STAGE < 10 * it + 5:
                    break
                nc.vector.tensor_tensor(
                    out=wvT[:].rearrange("p (c m) -> p c m", c=DC),
                    in0=wvp2[:].rearrange("p (c m) -> p c m", c=DC),
                    in1=sclbc_sb[:].unsqueeze(1).broadcast_to([128, DC, 64]),
                    op=OP.mult,
                )
                dump16(f"wvT_{it}", wvT[:], 128, DC * 64)

                # ---------- agreement[n, c] = x @ WvT ----------
                if STAGE < 10 * it + 6:
                    break
                for b in range(BL):
                    for t in range(NT):
                        for dc in range(DC):
                            rhs = diag_ap(
                                wvT, 0, dc * 64 + 2 * b, DC * 64,
                                [[DC * 64, 128], [4, NPAIR], [1, 2]],
                            )
                            nc.tensor.matmul(
                                agrp[b][:, t * C : (t + 1) * C],
                                xT_v[:, b, dc, t * 128 : (t + 1) * 128],
                                rhs,
                                start=(dc == 0 and t == 0 and it == 0),
                                stop=(dc == DC - 1 and t == NT - 1 and it == 1),
                                skip_group_check=True,
                            )
                dump32(f"agr_{it}", agrp[0][:], 128, NT * C)

            # ---------------- MHA on the routed capsules ----------------
            def _cut(n):
                if STAGE < n:
                    zt = kw.tile([64, 64], f32, tag="cut_z")
                    nc.gpsimd.memset(zt[:], 0.0)
                    nc.sync.dma_start(
                        out_d.ap().rearrange("b c h -> (b c) h"), zt[:]
                    )
                    raise _StageCut()
            _cut(50)

            # v in pair-packed partitions: v128[64i+h, 32b+2q+i] = v[b,2q+i,h]
            for i in range(2):
                dst = diag_ap(v128, 64 * i, i, 64, [[64, 64], [2, NPAIR], [32, 2]])
                sclbc_d = diag_ap(sclbc_sb, 64 * i, i, 64, [[64, 64], [4, NPAIR], [2, 2]])
                nc.vector.tensor_tensor(
                    out=dst, in0=su3_diag(i), in1=sclbc_d, op=OP.mult
                )
            dump16("v128", v128[:], 128, 64)

            _cut(51)
            # q/k projections: pqk[kd, (which,hd) x (b,c)]
            pqk = pp.tile([64, 512], f32, tag="xcpd0", name="pqk")
            for idx in range(8):
                nc.tensor.matmul(
                    pqk[:, idx * 64 : (idx + 1) * 64],
                    wqk2_sb[:, idx * 64 : (idx + 1) * 64],
                    v128[:],
                )
            # vbc[32b+c, h] for the residual: PE transpose + fold the
            # i-halves; emitted after pqk so it doesn't delay the q/k path.
            tpv = pt.tile([64, 128], bf16, tag="pt", name="tpv")
            nc.tensor.transpose(tpv[:], v128[:], i128b[:])
            tpv_sb = kw.tile([64, 128], f32, tag="tpv_sb")
            nc.vector.tensor_copy(tpv_sb[:], tpv[:])
            nc.vector.tensor_add(vbc[:], tpv_sb[:, 0:64], tpv_sb[:, 64:128])
            dump32("vbc", vbc[:], 64, 64)
            qk_sb = kw.tile([64, 512], bf16, tag="qk_sb")
            nc.vector.tensor_copy(qk_sb[:], pqk[:])
            dump16("qk", qk_sb[:], 64, 512)

            # v projection: vap[c, (hd,kd)] per batch
            vap = pp.tile([32, 512], f32, tag="xcpd1", name="vap")
            for b in range(BL):
                nc.tensor.matmul(
                    vap[:, b * 256 : (b + 1) * 256],
                    v128[:, b * 32 : (b + 1) * 32],
                    wvv2_sb[:],
                )
            vab_sb = kw.tile([32, 512], bf16, tag="vab_sb")
            nc.vector.tensor_copy(vab_sb[:], vap[:])

            _cut(52)
            # transposed scores: scp[kc, hd*64 + b*32 + qc]
            scp = pp.tile([32, 256], f32, tag="su3", name="scp")
            for hd in range(HEADS):
                for b in range(BL):
                    nc.tensor.matmul(
                        scp[:, hd * 64 + b * 32 : hd * 64 + b * 32 + 32],
                        qk_sb[:, 256 + hd * 64 + b * 32 : 256 + hd * 64 + b * 32 + 32],
                        qk_sb[:, hd * 64 + b * 32 : hd * 64 + b * 32 + 32],
                    )
            _cut(53)
            att_eT = kw.tile([32, 256], bf16, tag="att_eT")
            nc.scalar.activation(
                att_eT[:], scp[:], AF.Exp, scale=1.0 / float(np.sqrt(KD))
            )
            # softmax denominators via PE column-sum, then K=1 broadcast
            zp = pt.tile([1, 256], f32, tag="pt", name="zp")
            nc.tensor.matmul(zp[:], ones_b16[0:32, :], att_eT[:])
            rz = kw.tile([1, 256], f32, tag="rz")
            nc.vector.reciprocal(rz[:], zp[:])
            rzbc = pt.tile([32, 256], f32, tag="pt", name="rzbc")
            nc.tensor.matmul(rzbc[:], ones_row[:, 0:32], rz[:])
            attnT = kw.tile([32, 256], bf16, tag="attnT")
            nc.vector.tensor_tensor(
                out=attnT[:], in0=att_eT[:], in1=rzbc[:], op=OP.mult
            )
            dump16("attnT", attnT[:], 32, 256)
            _cut(54)
            # context, transposed: ctxp[kd, hd*64 + b*32 + qc]
            ctxp = pp.tile([64, 256], f32, tag="wvp2", name="ctxp")
            for hd in range(HEADS):
                for b in range(BL):
                    nc.tensor.matmul(
                        ctxp[:, hd * 64 + b * 32 : hd * 64 + b * 32 + 32],
                        vab_sb[:, b * 256 + hd * 64 : b * 256 + (hd + 1) * 64],
                        attnT[:, hd * 64 + b * 32 : hd * 64 + b * 32 + 32],
                    )
            ctx_sb = kw.tile([64, 256], bf16, tag="ctx_sb")
            nc.vector.tensor_copy(ctx_sb[:], ctxp[:])
            dump16("ctx", ctx_sb[:], 64, 256)
            _cut(55)
            # output projection, contract (hd,kd): mham[(b,qc), h]
            mham = pp.tile([64, 64], f32, tag="agrp0", name="mham")
            for hd in range(HEADS):
                nc.tensor.matmul(
                    mham[:],
                    ctx_sb[:, hd * 64 : (hd + 1) * 64],
                    wob2_sb[:, hd * 64 : (hd + 1) * 64],
                    start=(hd == 0),
                    stop=(hd == HEADS - 1),
                )
            y = kw.tile([64, 64], f32, tag="y")
            nc.vector.tensor_add(y[:], mham[:], vbc[:])
            dump32("y", y[:], 64, 64)

            _cut(56)
            # layernorm + final squash, fused into one per-row scale:
            #   zl = var + eps;  n2 = sum(ln^2) = H*var/zl  (closed form)
            #   out = yc * gamma*n2/(1+n2) * rsqrt(zl*(n2+eps_sq))
            mu = kw.tile([64, 1], f32, tag="mu")
            ysc = kw.tile([64, 64], f32, tag="ysc")
            nc.vector.tensor_scalar(
                out=ysc[:], in0=y[:], scalar1=1.0 / H, scalar2=0.0,
                op0=OP.mult, op1=OP.add, accum_out=mu[:],
            )
            yc = kw.tile([64, 64], f32, tag="yc")
            nc.vector.tensor_scalar(
                out=yc[:], in0=y[:], scalar1=mu[:], scalar2=None,
                op0=OP.subtract,
            )
            sqy = kw.tile([64, 64], f32, tag="sqy")
            var_r = kw.tile([64, 1], f32, tag="var_r")
            nc.scalar.activation(sqy[:], yc[:], AF.Square, accum_out=var_r[:])
            zl = kw.tile([64, 1], f32, tag="zl")
            nc.vector.tensor_scalar(
                out=zl[:], in0=var_r[:], scalar1=1.0 / H, scalar2=LN_EPS,
                op0=OP.mult, op1=OP.add,
            )
            _cut(57)
            b0 = kw.tile([64, 1], f32, tag="b0")
            nc.vector.reciprocal(b0[:], zl[:])
            n2 = kw.tile([64, 1], f32, tag="n2")
            nc.vector.tensor_mul(n2[:], b0[:], var_r[:])
            pf = kw.tile([64, 1], f32, tag="pf")
            nc.vector.scalar_tensor_tensor(
                out=pf[:], in0=n2[:], scalar=EPS_SQ, in1=zl[:],
                op0=OP.add, op1=OP.mult,
            )
            yq = quake_rsqrt(pf[:], "fin", 64)
            b1 = kw.tile([64, 1], f32, tag="b1")
            nc.vector.tensor_scalar_add(b1[:], n2[:], 1.0)
            b2 = kw.tile([64, 1], f32, tag="b2")
            nc.vector.reciprocal(b2[:], b1[:])
            gf = kw.tile([64, 1], f32, tag="gf")
            nc.vector.scalar_tensor_tensor(
                out=gf[:], in0=n2[:], scalar=float(gamma_val), in1=b2[:],
                op0=OP.mult, op1=OP.mult,
            )
            sclf = kw.tile([64, 1], f32, tag="sclf")
            nc.vector.tensor_mul(sclf[:], gf[:], yq[:])
            outf = kw.tile([64, 64], f32, tag="outf")
            nc.vector.tensor_scalar_mul(outf[:], yc[:], sclf[:])
            nc.sync.dma_start(out_d.ap().rearrange("b c h -> (b c) h"), outf[:])

      except _StageCut:
        pass
    nc.compile()
    return nc


def _prep_inputs(inputs):
    x = np.asarray(inputs["x"], np.float32)
    W = np.asarray(inputs["W"], np.float32)
    b_caps = np.asarray(inputs["b_caps"], np.float32)
    gamma = np.asarray(inputs["gamma"], np.float32)
    Wq = np.asarray(inputs["Wq"], np.float32)
    Wk = np.asarray(inputs["Wk"], np.float32)
    Wv = np.asarray(inputs["Wv"], np.float32)
    Wo = np.asarray(inputs["Wo"], np.float32)
    bq = np.asarray(inputs["bq"], np.float32)
    bk = np.asarray(inputs["bk"], np.float32)
    bv = np.asarray(inputs["bv"], np.float32)
    bo = np.asarray(inputs["bo"], np.float32)
    ln_gamma = np.asarray(inputs["ln_gamma"], np.float32)
    ln_beta = np.asarray(inputs["ln_beta"], np.float32)

    # this kernel implements the zero-bias / identity-LN configuration the
    # problem's setup_inputs() produces; fail loudly on anything else.
    assert not np.any(b_caps), "b_caps != 0 unsupported"
    assert not (np.any(bq) or np.any(bk) or np.any(bv) or np.any(bo)), \
        "attention biases != 0 unsupported"
    assert np.all(ln_gamma == 1.0) and not np.any(ln_beta), \
        "non-identity layernorm affine unsupported"

    bf16 = ml_dtypes.bfloat16
    # n-major x, partition-major host layout [core, p, b, t, d]
    xn = np.ascontiguousarray(
        x.reshape(NCORES, BL, NT, 128, D).transpose(0, 3, 1, 2, 4)
    ).astype(bf16)
    # d-major x, partition-major [core, p(d'), b, dc, n]
    xt = np.ascontiguousarray(
        x.reshape(NCORES, BL, N, DC, 128).transpose(0, 4, 1, 3, 2)
    ).astype(bf16)
    # xbar[b, d] = sum_n x[b, n, d], laid out [core, p(d'), b*DC+dc]
    xbar = x.reshape(NCORES, BL, N, DC, 128).sum(axis=2)  # [r, b, dc, 128]
    xbarT = np.ascontiguousarray(xbar.transpose(0, 3, 1, 2)).reshape(
        NCORES, 128, BL * DC
    ).astype(np.float32)
    # W for the s-matmul: ws[d', q, dc, 64i+h] = W[2q+i, dc*128+d', h]
    ws = np.ascontiguousarray(
        W.reshape(NPAIR, 2, DC, 128, H).transpose(3, 0, 2, 1, 4)
    ).reshape(128, NPAIR * DC * 128).astype(bf16)
    # W for the Wv-matmul: wvw[64i+h, q, dc, d'] = W[2q+i, dc*128+d', h]
    wvw = np.ascontiguousarray(
        W.reshape(NPAIR, 2, DC, 128, H).transpose(1, 4, 0, 2, 3)
    ).reshape(128, NPAIR * DC * 128).astype(bf16)

    # MHA weights, replicated over the two i-halves of the partition dim
    wqk1 = np.concatenate(
        [Wq.reshape(H, HEADS * KD), Wk.reshape(H, HEADS * KD)], axis=1
    )
    wqk2 = np.ascontiguousarray(np.tile(wqk1, (2, 1))).astype(bf16)
    wvv2 = np.ascontiguousarray(
        np.tile(Wv.reshape(H, HEADS * KD), (2, 1))
    ).astype(bf16)
    wob2 = np.ascontiguousarray(
        Wo.transpose(1, 0, 2).reshape(KD, HEADS * H)
    ).astype(bf16)

    common = dict(
        ws=ws,
        wvw=wvw,
        wqk2=wqk2,
        wvv2=wvv2,
        wob2=wob2,
        i128b=np.eye(128, dtype=bf16),
    )
    in_maps = []
    for r in range(NCORES):
        m = dict(common)
        m["xn"] = xn[r]
        m["xt"] = xt[r]
        m["xbar"] = xbarT[r]
        in_maps.append(m)
    return in_maps, float(gamma.reshape(-1)[0])


def _run(inputs, trace=False):
    from concourse.bass_utils import run_bass_kernel_spmd

    in_maps, gamma_val = _prep_inputs(inputs)
    key = gamma_val
    if key not in _CACHE:
        _CACHE[key] = _build(gamma_val)
    nc = _CACHE[key]
    res = run_bass_kernel_spmd(
        nc, in_maps, core_ids=list(range(NCORES)), trace=trace
    )
    out = np.concatenate(
        [np.asarray(res.results[r]["out"]) for r in range(NCORES)], axis=0
    ).astype(np.float32)
    return out, res


def kernel(**inputs):
    out, _ = _run(inputs, trace=False)
    return out
